# revision 19
# baseline (speedup 1.0000x reference)
"""GAT layer kernel for Trainium2, 8 NeuronCores, batch-sharded.

Math (per graph g of B=128, M=512 nodes, in=128, out D=64):
    Wh = h @ W.T;  s_src = Wh @ a[:D];  s_dst = Wh @ a[D:]
    e[i,j] = leakyrelu_0.2(s_src[i] + s_dst[j])
    out = elu(softmax(e, -1) @ Wh)

Device formulation (per core: 16 graphs). All matmuls bf16. Host folds a
into the weight, Wq = [W.T | W.T@a_dst | W.T@a_src], and pre-transposes
h -> hT [16, 128k, 512m]. Per graph:
  - 4 matmuls  lhsT=hT chunk, rhs=Wq -> psum [n, 66] = [Wh | s_dst | s_src];
    cast to SBUF bf16 with a ones column appended -> WhA [n, 67].
  - 1 matmul   lhsT=wsb (host constant, every column w_src):
    E0[n, m] = s_src[m] broadcast directly from hT (psum).
  - ACT Prelu per chunk: q = lrelu(K1*(E0 + s_dst[n])) = K1*lrelu(e)
    with scale=K1=128*log2(e) and bias=K1*s_dst riding in the activation
    (fp32 out). This is the only PSUM->SBUF drain of the M^2 tensor.
  - DVE fast-exp (Schraudolph): ONE tensor_scalar converts q fp32 ->
    int16 with +K2 (K2 = 127*128 - c); the int16 bit pattern IS the bf16
    encoding of 2^(lrelu(e)*log2 e) = exp(lrelu(e)). Bitcast to bf16 ->
    softmax numerator P at 4x-ish DVE rate, no ACT Exp pass at all.
    Softmax max-subtraction is skipped: |e| < ~10, exp cannot overflow,
    and softmax is shift-invariant.
  - 16 matmuls lhsT=P chunks, rhs=WhA accumulating psum [m, 67]: cols
    0:64 = unnormalized A@Wh, col 66 = softmax denominator Z_m.
  - Final (pair-batched over 2 graphs to amortize op overheads):
    r=1/Z; elu(x*r) = (relu(x)*r - 1) + exp(min(x,0)*r) via two fused
    scalar_tensor_tensor ops + one small exact ACT Exp.
"""

import os
import sys
import types
from contextlib import ExitStack

import numpy as np
import ml_dtypes

# Defensive: concourse.bass_utils imports antenv.axon_hooks when tracing is
# requested (BASS_TRACE). Some images lack that module; register a stub so a
# traced run degrades to untraced instead of crashing.
try:
    import antenv.axon_hooks  # noqa: F401
except Exception:
    try:
        import antenv

        _hooks = types.ModuleType("antenv.axon_hooks")
        _hooks._hook = None
        _hooks.set_axon_ntff_profile_hook = lambda h: setattr(_hooks, "_hook", h)
        _hooks.get_axon_ntff_profile_hook = lambda: _hooks._hook
        sys.modules["antenv.axon_hooks"] = _hooks
        antenv.axon_hooks = _hooks
    except Exception:
        pass

import concourse.bass as bass
import concourse.tile as tile
from concourse import bacc, mybir
from concourse import dve_ops as _dve_ops
from concourse._compat import with_exitstack
from concourse.bass import ds, ts
from concourse.bass_utils import run_bass_kernel_spmd
from concourse.dve_spec import (
    C0 as _C0,
    C1 as _C1,
    C2 as _C2,
    C3 as _C3,
    Spec as _Spec,
    Src0 as _Src0,
    _spill_c3_to_src1,
    lower as _dve_lower,
    maxx as _maxx,
)
from concourse.dve_uop import DveOpSpec as _DveOpSpec


def _register_fexp_op():
    """Custom DVE op: out_i16 = int16(max((x+c0)*K1, (x+c3)*0.2K1)).

    With c0 = sd + K2/K1 and c3 = sd + K2/(0.2*K1) this equals
    int16(lrelu(x + sd)*K1 + K2) -- the whole fused bias + leakyrelu +
    Schraudolph-affine in ONE DVE instruction reading PSUM."""
    name = "FEXP_LRELU_ANT"
    for op in _dve_ops.OPS:
        if op.name == name:
            return op
    body = _maxx((_Src0 + _C0) * _C1, (_Src0 + _C3) * _C2)
    spec = _Spec(
        body=_spill_c3_to_src1(body),
        reference=lambda in0, in1, s0, s1, imm2: np.maximum(
            (in0 + s0) * s1, (in0 + in1) * imm2
        ),
    )
    opcode = _dve_ops._CUSTOM_DVE_ROW_BASE + len(_dve_ops.OPS)
    shas = {}
    for ver in ("v3", "v4"):
        shas[ver] = _DveOpSpec(
            name=name, opcode=opcode, uops=_dve_lower(spec, ver=ver), rd1_en=True
        ).sha(ver)
    op = _dve_ops.DveOp(name, spec, subdim=False, uops_sha=shas)
    _dve_ops.OPS.append(op)
    _dve_ops._SUB_OPCODE_FOR_NAME[name] = opcode
    return op


_FEXP_OP = _register_fexp_op()

B, M, IN_DIM, D = 128, 512, 128, 64
N_CORES = 8
G = B // N_CORES  # graphs per core
NC = M // 128  # 128-node chunks per graph
ALPHA = 0.2
F32 = mybir.dt.float32
BF16 = mybir.dt.bfloat16
I16 = mybir.dt.int16

# Schraudolph fast-exp constants (bf16 variant):
# bits_i16 = round(lrelu(e) * K1) + K2; bitcast bf16 ~= exp(lrelu(e)).
K1 = 184.6650390625  # 128 * log2(e)
K2I = 16249  # 127*128 minus centering constant c=7 (integer add)
C0K = 87.99175026573663  # K2/K1
C3K = 439.95875132868315  # K2/(0.2*K1)
A1K = 36.9330078125  # 0.2*K1

LAST_RESULTS = None  # BassKernelResults of the most recent run (for test.py)


@with_exitstack
def _gat_body(ctx: ExitStack, tc: tile.TileContext, out_ap, ht_ap, wq_ap, wsb_ap):
    nc = tc.nc
    const = ctx.enter_context(tc.tile_pool(name="const", bufs=1))
    ht_pool = ctx.enter_context(tc.tile_pool(name="ht", bufs=5))
    wha_pool = ctx.enter_context(tc.tile_pool(name="wha", bufs=5))
    q_pool = ctx.enter_context(tc.tile_pool(name="q", bufs=3))
    p_pool = ctx.enter_context(tc.tile_pool(name="p", bufs=3))
    fin_pool = ctx.enter_context(tc.tile_pool(name="fin", bufs=3))
    out_pool = ctx.enter_context(tc.tile_pool(name="out", bufs=3))
    ps_wh = ctx.enter_context(tc.tile_pool(name="ps_wh", bufs=1, space="PSUM"))
    ps_e = ctx.enter_context(tc.tile_pool(name="ps_e", bufs=3, space="PSUM"))
    ps_o = ctx.enter_context(tc.tile_pool(name="ps_o", bufs=2, space="PSUM"))

    wq_s = const.tile([IN_DIM, D + 1], BF16)
    nc.sync.dma_start(wq_s[:], wq_ap[:])

    # wsb[k, n] = w_src[k] (host constant, all columns identical): one
    # matmul lhsT=wsb, rhs=hT gives E0[n, m] = s_src[m] directly.
    wsb_s = const.tile([IN_DIM, 128], BF16)
    nc.sync.dma_start(wsb_s[:], wsb_ap[:])

    # Dummy activation at kernel start: triggers the one-time ~2.7us
    # exp_and_others ACT table load while the first ht DMA is in flight.
    warm = const.tile([1, 16], F32)
    nc.vector.memset(warm[:], 0.0)
    nc.scalar.activation(warm[:], warm[:], mybir.ActivationFunctionType.Prelu, alpha=ALPHA)

    p_o_pair = None
    for g in range(G):
        ht_s = ht_pool.tile([IN_DIM, M], BF16)
        nc.sync.dma_start(ht_s[:], ht_ap[g])

        # Wh + score columns for each node chunk: psum [128n, 66]
        p_wh = ps_wh.tile([128, NC, D + 1], F32)
        for c in range(NC):
            nc.tensor.matmul(
                p_wh[:, c, :], ht_s[:, ts(c, 128)], wq_s[:], start=True, stop=True
            )
        # WhA [n, 66] = [Wh | s_dst | 1]  (bf16)
        wha = wha_pool.tile([128, NC, D + 2], BF16)
        nc.vector.tensor_copy(wha[:, :, 0 : D + 1], p_wh[:, :, :])
        if g < 5:
            # ones column: written once per pool buffer (5 bufs, round-
            # robin); later graphs reuse the already-set column.
            nc.gpsimd.memset(wha[:, :, D + 1 : D + 2], 1.0)
        # biasK[n, c] = K1 * s_dst (fp32): the Prelu per-partition bias.
        # Computed on GpSimd (from the bf16 sd column -- +-0.5% exp
        # ripple, immaterial) to keep it off the busy DVE queue.
        biasK = fin_pool.tile([128, NC, 1], F32, tag="biasK")
        nc.gpsimd.tensor_scalar(
            biasK[:], wha[:, :, D : D + 1], K1, None, mybir.AluOpType.mult
        )

        # E0[n, m] = s_src[m]: one matmul, every lhsT column is w_src
        p_e0 = ps_e.tile([128, M], F32)
        nc.tensor.matmul(p_e0[:], wsb_s[:], ht_s[:], start=True, stop=True)

        # q = int16(K1 * lrelu(E0 + s_dst)) per chunk (= lrelu(K1*E0 +
        # K1*sd), valid since K1 > 0). ACT converts fp32->int16 at
        # writeback; int16 keeps full mantissa for the exponent bits.
        # On odd graphs the last chunk goes to the DVE instead via the
        # fused custom op (ACT/DVE load balancing, k = 3.5 avg).
        n_act = NC if g % 2 == 0 else NC - 1
        q_t = q_pool.tile([128, NC, M], I16)
        p_i = p_pool.tile([128, NC, M], I16)
        for c in range(n_act):
            nc.scalar.activation(
                q_t[:, c, :],
                p_e0[:],
                mybir.ActivationFunctionType.Prelu,
                bias=biasK[:, c, :],
                scale=K1,
                alpha=ALPHA,
            )
        if n_act < NC:
            c03 = fin_pool.tile([128, NC, 2], F32, tag="c03")
            nc.gpsimd.tensor_scalar(
                c03[:, :, 0:1], wha[:, :, D : D + 1], C0K, None,
                mybir.AluOpType.add,
            )
            nc.gpsimd.tensor_scalar(
                c03[:, :, 1:2], wha[:, :, D : D + 1], C3K, None,
                mybir.AluOpType.add,
            )
            for c in range(n_act, NC):
                nc.vector._custom_dve(
                    _FEXP_OP,
                    out=p_i[:, c, :],
                    in0=p_e0[:],
                    in1=c03[:, c, 1:2],
                    s0=c03[:, c, 0:1],
                    s1=K1,
                    imm2=A1K,
                )
        # Fast-exp: int16 bits += K2 (integer add, 16-bit 4x DVE mode),
        # bitcast as bf16 -> P
        nc.vector.tensor_scalar(
            p_i[:, 0:n_act, :], q_t[:, 0:n_act, :], K2I, None,
            mybir.AluOpType.add,
        )
        p_t = p_i[:].bitcast(BF16)

        # attention: psum [m, 67]; col 66 = Z_m. Pair-batched psum tile,
        # padded to 128 floats per (half, mc) block so every matmul
        # output stays inside one 2KB PSUM bank.
        half = g % 2
        if half == 0:
            p_o_pair = ps_o.tile([128, 2, NC, 128], F32)

        def emit_final_pair():
            # final over both graphs: elu(x*r) = (relu(x)*r - 1) +
            # exp(min(x,0)*r), r = 1/Z
            r4 = fin_pool.tile([128, 2, NC], F32, tag="r4")
            nc.vector.reciprocal(r4[:], p_o_pair[:, :, :, D + 1])
            r4b = r4[:].unsqueeze(3).broadcast_to([128, 2, NC, D])
            a_t = fin_pool.tile([128, 2, NC, D], F32, tag="a")
            nc.vector.scalar_tensor_tensor(
                a_t[:], p_o_pair[:, :, :, 0:D], 0.0, r4b,
                mybir.AluOpType.min, mybir.AluOpType.mult,
            )
            v_t = fin_pool.tile([128, 2, NC, D], F32, tag="v")
            nc.scalar.activation(
                v_t[:], a_t[:], mybir.ActivationFunctionType.Exp
            )
            b_t = fin_pool.tile([128, 2, NC, D], F32, tag="b")
            nc.vector.scalar_tensor_tensor(
                b_t[:], p_o_pair[:, :, :, 0:D], 0.0, r4b,
                mybir.AluOpType.max, mybir.AluOpType.mult,
            )
            o_t = out_pool.tile([128, 2, NC, D], F32, tag="o")
            nc.vector.scalar_tensor_tensor(
                o_t[:], b_t[:], -1.0, v_t[:],
                mybir.AluOpType.add, mybir.AluOpType.add,
            )
            dst = out_ap[ds(g - 1, 2)].transpose([2, 0, 1, 3])
            nc.sync.dma_start(dst, o_t[:])

        def emit_final_one(hh, gg):
            # final for one graph of the pair (tail-drain trim)
            r4 = fin_pool.tile([128, 2, NC], F32, tag="r4")
            nc.vector.reciprocal(r4[:, 0, :], p_o_pair[:, hh, :, D + 1])
            r4b = r4[:, 0, :].unsqueeze(2).broadcast_to([128, NC, D])
            a_t = fin_pool.tile([128, 2, NC, D], F32, tag="a")
            nc.vector.scalar_tensor_tensor(
                a_t[:, 0], p_o_pair[:, hh, :, 0:D], 0.0, r4b,
                mybir.AluOpType.min, mybir.AluOpType.mult,
            )
            v_t = fin_pool.tile([128, 2, NC, D], F32, tag="v")
            nc.scalar.activation(
                v_t[:, 0], a_t[:, 0], mybir.ActivationFunctionType.Exp
            )
            b_t = fin_pool.tile([128, 2, NC, D], F32, tag="b")
            nc.vector.scalar_tensor_tensor(
                b_t[:, 0], p_o_pair[:, hh, :, 0:D], 0.0, r4b,
                mybir.AluOpType.max, mybir.AluOpType.mult,
            )
            o_t = out_pool.tile([128, 2, NC, D], F32, tag="o")
            nc.vector.scalar_tensor_tensor(
                o_t[:, 0], b_t[:, 0], -1.0, v_t[:, 0],
                mybir.AluOpType.add, mybir.AluOpType.add,
            )
            dst = out_ap[gg].transpose([1, 0, 2])
            nc.sync.dma_start(dst, o_t[:, 0])

        for mc in range(NC):
            for c in range(NC):
                nc.tensor.matmul(
                    p_o_pair[:, half, mc, 0 : D + 2],
                    p_t[:, c, ds(mc * 128, 128)],
                    wha[:, c, :],
                    start=(c == 0),
                    stop=(c == NC - 1),
                )
        # Last pair: per-graph finals so graph G-2's ELU/DMA overlaps
        # graph G-1's attention matmuls (tail-drain trim).
        if g >= G - 2:
            emit_final_one(half, g)
        elif half == 1:
            emit_final_pair()


_CACHE = {}


def _build():
    if "nc" in _CACHE:
        return _CACHE["nc"]
    nc = bacc.Bacc(
        "TRN2", target_bir_lowering=False, debug=False, num_devices=N_CORES
    )
    ht_d = nc.dram_tensor("ht", [G, IN_DIM, M], BF16, kind="ExternalInput")
    wq_d = nc.dram_tensor("wq", [IN_DIM, D + 1], BF16, kind="ExternalInput")
    wsb_d = nc.dram_tensor("wsb", [IN_DIM, 128], BF16, kind="ExternalInput")
    # [G, NC, 128, D] has the same memory layout as [G, M, D] (M = NC*128)
    out_d = nc.dram_tensor("out", [G, NC, 128, D], F32, kind="ExternalOutput")
    with tile.TileContext(nc) as tc:
        _gat_body(tc, out_d.ap(), ht_d.ap(), wq_d.ap(), wsb_d.ap())
    nc.compile()
    _CACHE["nc"] = nc
    return nc


def kernel(h, W, a):
    global LAST_RESULTS
    h = np.asarray(h, dtype=np.float32)
    W = np.asarray(W, dtype=np.float32)
    a = np.asarray(a, dtype=np.float32)

    wt = W.T.astype(np.float32)  # [128, 64]
    wq = np.concatenate(
        [wt, (wt @ a[D:])[:, None]], axis=1
    ).astype(ml_dtypes.bfloat16)  # [128, 65] = [W.T | w_dst]
    w_src = (wt @ a[:D]).astype(ml_dtypes.bfloat16)  # [128]
    wsb = np.ascontiguousarray(np.repeat(w_src[:, None], 128, axis=1))

    nc = _build()
    in_maps = []
    for c in range(N_CORES):
        h_c = h[c * G : (c + 1) * G]  # [G, 512, 128]
        ht_c = np.ascontiguousarray(h_c.transpose(0, 2, 1)).astype(
            ml_dtypes.bfloat16
        )  # [G, 128, 512]
        in_maps.append({"ht": ht_c, "wq": wq, "wsb": wsb})

    res = run_bass_kernel_spmd(nc, in_maps, list(range(N_CORES)))
    LAST_RESULTS = res
    out = np.concatenate(
        [r["out"].reshape(G, M, D) for r in res.results], axis=0
    )
    return out.astype(np.float32)


# revision 20
# speedup vs baseline: 1.0067x; 1.0067x over previous
"""GAT layer kernel for Trainium2, 8 NeuronCores, batch-sharded.

Math (per graph g of B=128, M=512 nodes, in=128, out D=64):
    Wh = h @ W.T;  s_src = Wh @ a[:D];  s_dst = Wh @ a[D:]
    e[i,j] = leakyrelu_0.2(s_src[i] + s_dst[j])
    out = elu(softmax(e, -1) @ Wh)

Device formulation (per core: 16 graphs). All matmuls bf16. Host folds a
into the weight, Wq = [W.T | W.T@a_dst | W.T@a_src], and pre-transposes
h -> hT [16, 128k, 512m]. Per graph:
  - 4 matmuls  lhsT=hT chunk, rhs=Wq -> psum [n, 66] = [Wh | s_dst | s_src];
    cast to SBUF bf16 with a ones column appended -> WhA [n, 67].
  - 1 matmul   lhsT=wsb (host constant, every column w_src):
    E0[n, m] = s_src[m] broadcast directly from hT (psum).
  - ACT Prelu per chunk: q = lrelu(K1*(E0 + s_dst[n])) = K1*lrelu(e)
    with scale=K1=128*log2(e) and bias=K1*s_dst riding in the activation
    (fp32 out). This is the only PSUM->SBUF drain of the M^2 tensor.
  - DVE fast-exp (Schraudolph): ONE tensor_scalar converts q fp32 ->
    int16 with +K2 (K2 = 127*128 - c); the int16 bit pattern IS the bf16
    encoding of 2^(lrelu(e)*log2 e) = exp(lrelu(e)). Bitcast to bf16 ->
    softmax numerator P at 4x-ish DVE rate, no ACT Exp pass at all.
    Softmax max-subtraction is skipped: |e| < ~10, exp cannot overflow,
    and softmax is shift-invariant.
  - 16 matmuls lhsT=P chunks, rhs=WhA accumulating psum [m, 67]: cols
    0:64 = unnormalized A@Wh, col 66 = softmax denominator Z_m.
  - Final (pair-batched over 2 graphs to amortize op overheads):
    r=1/Z; elu(x*r) = (relu(x)*r - 1) + exp(min(x,0)*r) via two fused
    scalar_tensor_tensor ops + one small exact ACT Exp.
"""

import os
import sys
import types
from contextlib import ExitStack

import numpy as np
import ml_dtypes

# Defensive: concourse.bass_utils imports antenv.axon_hooks when tracing is
# requested (BASS_TRACE). Some images lack that module; register a stub so a
# traced run degrades to untraced instead of crashing.
try:
    import antenv.axon_hooks  # noqa: F401
except Exception:
    try:
        import antenv

        _hooks = types.ModuleType("antenv.axon_hooks")
        _hooks._hook = None
        _hooks.set_axon_ntff_profile_hook = lambda h: setattr(_hooks, "_hook", h)
        _hooks.get_axon_ntff_profile_hook = lambda: _hooks._hook
        sys.modules["antenv.axon_hooks"] = _hooks
        antenv.axon_hooks = _hooks
    except Exception:
        pass

import concourse.bass as bass
import concourse.tile as tile
from concourse import bacc, mybir
from concourse import dve_ops as _dve_ops
from concourse._compat import with_exitstack
from concourse.bass import ds, ts
from concourse.bass_utils import run_bass_kernel_spmd
from concourse.dve_spec import (
    C0 as _C0,
    C1 as _C1,
    C2 as _C2,
    C3 as _C3,
    Spec as _Spec,
    Src0 as _Src0,
    _spill_c3_to_src1,
    lower as _dve_lower,
    maxx as _maxx,
)
from concourse.dve_uop import DveOpSpec as _DveOpSpec


def _register_fexp_op():
    """Custom DVE op: out_i16 = int16(max((x+c0)*K1, (x+c3)*0.2K1)).

    With c0 = sd + K2/K1 and c3 = sd + K2/(0.2*K1) this equals
    int16(lrelu(x + sd)*K1 + K2) -- the whole fused bias + leakyrelu +
    Schraudolph-affine in ONE DVE instruction reading PSUM."""
    name = "FEXP_LRELU_ANT"
    for op in _dve_ops.OPS:
        if op.name == name:
            return op
    body = _maxx((_Src0 + _C0) * _C1, (_Src0 + _C3) * _C2)
    spec = _Spec(
        body=_spill_c3_to_src1(body),
        reference=lambda in0, in1, s0, s1, imm2: np.maximum(
            (in0 + s0) * s1, (in0 + in1) * imm2
        ),
    )
    opcode = _dve_ops._CUSTOM_DVE_ROW_BASE + len(_dve_ops.OPS)
    shas = {}
    for ver in ("v3", "v4"):
        shas[ver] = _DveOpSpec(
            name=name, opcode=opcode, uops=_dve_lower(spec, ver=ver), rd1_en=True
        ).sha(ver)
    op = _dve_ops.DveOp(name, spec, subdim=False, uops_sha=shas)
    _dve_ops.OPS.append(op)
    _dve_ops._SUB_OPCODE_FOR_NAME[name] = opcode
    return op


_FEXP_OP = _register_fexp_op()

B, M, IN_DIM, D = 128, 512, 128, 64
N_CORES = 8
G = B // N_CORES  # graphs per core
NC = M // 128  # 128-node chunks per graph
ALPHA = 0.2
F32 = mybir.dt.float32
BF16 = mybir.dt.bfloat16
I16 = mybir.dt.int16

# Schraudolph fast-exp constants (bf16 variant):
# bits_i16 = round(lrelu(e) * K1) + K2; bitcast bf16 ~= exp(lrelu(e)).
K1 = 184.6650390625  # 128 * log2(e)
K2I = 16249  # 127*128 minus centering constant c=7 (integer add)
C0K = 87.99175026573663  # K2/K1
C3K = 439.95875132868315  # K2/(0.2*K1)
A1K = 36.9330078125  # 0.2*K1

LAST_RESULTS = None  # BassKernelResults of the most recent run (for test.py)


@with_exitstack
def _gat_body(ctx: ExitStack, tc: tile.TileContext, out_ap, ht_ap, wq_ap, wsb_ap):
    nc = tc.nc
    const = ctx.enter_context(tc.tile_pool(name="const", bufs=1))
    ht_pool = ctx.enter_context(tc.tile_pool(name="ht", bufs=5))
    wha_pool = ctx.enter_context(tc.tile_pool(name="wha", bufs=5))
    q_pool = ctx.enter_context(tc.tile_pool(name="q", bufs=3))
    p_pool = ctx.enter_context(tc.tile_pool(name="p", bufs=3))
    fin_pool = ctx.enter_context(tc.tile_pool(name="fin", bufs=3))
    out_pool = ctx.enter_context(tc.tile_pool(name="out", bufs=3))
    ps_wh = ctx.enter_context(tc.tile_pool(name="ps_wh", bufs=1, space="PSUM"))
    ps_e = ctx.enter_context(tc.tile_pool(name="ps_e", bufs=3, space="PSUM"))
    ps_o = ctx.enter_context(tc.tile_pool(name="ps_o", bufs=2, space="PSUM"))

    wq_s = const.tile([IN_DIM, D + 1], BF16)
    nc.sync.dma_start(wq_s[:], wq_ap[:])

    # wsb[k, n] = w_src[k] (host constant, all columns identical): one
    # matmul lhsT=wsb, rhs=hT gives E0[n, m] = s_src[m] directly.
    wsb_s = const.tile([IN_DIM, 128], BF16)
    nc.sync.dma_start(wsb_s[:], wsb_ap[:])

    # Dummy activation at kernel start: triggers the one-time ~2.7us
    # exp_and_others ACT table load while the first ht DMA is in flight.
    warm = const.tile([1, 16], F32)
    nc.vector.memset(warm[:], 0.0)
    nc.scalar.activation(warm[:], warm[:], mybir.ActivationFunctionType.Prelu, alpha=ALPHA)

    p_o_pair = None
    for g in range(G):
        ht_s = ht_pool.tile([IN_DIM, M], BF16)
        nc.sync.dma_start(ht_s[:], ht_ap[g])

        # Wh + score columns for each node chunk: psum [128n, 66]
        p_wh = ps_wh.tile([128, NC, D + 1], F32)
        for c in range(NC):
            nc.tensor.matmul(
                p_wh[:, c, :], ht_s[:, ts(c, 128)], wq_s[:], start=True, stop=True
            )
        # biasK[n, c] = K1 * s_dst (fp32): the Prelu per-partition bias.
        # Issued FIRST on DVE straight from PSUM -- the prelus block on
        # it, so it must not queue behind bigger DVE ops or hop engines.
        biasK = fin_pool.tile([128, NC, 1], F32, tag="biasK")
        nc.vector.tensor_scalar(
            biasK[:], p_wh[:, :, D : D + 1], K1, None, mybir.AluOpType.mult
        )
        # WhA [n, 66] = [Wh | s_dst | 1]  (bf16)
        wha = wha_pool.tile([128, NC, D + 2], BF16)
        nc.vector.tensor_copy(wha[:, :, 0 : D + 1], p_wh[:, :, :])
        if g < 5:
            # ones column: written once per pool buffer (5 bufs, round-
            # robin); later graphs reuse the already-set column.
            nc.gpsimd.memset(wha[:, :, D + 1 : D + 2], 1.0)

        # E0[n, m] = s_src[m]: one matmul, every lhsT column is w_src
        p_e0 = ps_e.tile([128, M], F32)
        nc.tensor.matmul(p_e0[:], wsb_s[:], ht_s[:], start=True, stop=True)

        # q = int16(K1 * lrelu(E0 + s_dst)) per chunk (= lrelu(K1*E0 +
        # K1*sd), valid since K1 > 0). ACT converts fp32->int16 at
        # writeback; int16 keeps full mantissa for the exponent bits.
        # On odd graphs the last chunk goes to the DVE instead via the
        # fused custom op (ACT/DVE load balancing, k = 3.5 avg).
        n_act = NC if g % 4 != 3 else NC - 1
        q_t = q_pool.tile([128, NC, M], I16)
        p_i = p_pool.tile([128, NC, M], I16)
        for c in range(n_act):
            nc.scalar.activation(
                q_t[:, c, :],
                p_e0[:],
                mybir.ActivationFunctionType.Prelu,
                bias=biasK[:, c, :],
                scale=K1,
                alpha=ALPHA,
            )
        if n_act < NC:
            c03 = fin_pool.tile([128, NC, 2], F32, tag="c03")
            nc.vector.tensor_scalar(
                c03[:, :, 0:1], p_wh[:, :, D : D + 1], C0K, None,
                mybir.AluOpType.add,
            )
            nc.vector.tensor_scalar(
                c03[:, :, 1:2], p_wh[:, :, D : D + 1], C3K, None,
                mybir.AluOpType.add,
            )
            for c in range(n_act, NC):
                nc.vector._custom_dve(
                    _FEXP_OP,
                    out=p_i[:, c, :],
                    in0=p_e0[:],
                    in1=c03[:, c, 1:2],
                    s0=c03[:, c, 0:1],
                    s1=K1,
                    imm2=A1K,
                )
        # Fast-exp: int16 bits += K2 (integer add, 16-bit 4x DVE mode),
        # bitcast as bf16 -> P
        nc.vector.tensor_scalar(
            p_i[:, 0:n_act, :], q_t[:, 0:n_act, :], K2I, None,
            mybir.AluOpType.add,
        )
        p_t = p_i[:].bitcast(BF16)

        # attention: psum [m, 67]; col 66 = Z_m. Pair-batched psum tile,
        # padded to 128 floats per (half, mc) block so every matmul
        # output stays inside one 2KB PSUM bank.
        half = g % 2
        if half == 0:
            p_o_pair = ps_o.tile([128, 2, NC, 128], F32)

        def emit_final_pair():
            # final over both graphs: elu(x*r) = (relu(x)*r - 1) +
            # exp(min(x,0)*r), r = 1/Z
            r4 = fin_pool.tile([128, 2, NC], F32, tag="r4")
            nc.vector.reciprocal(r4[:], p_o_pair[:, :, :, D + 1])
            r4b = r4[:].unsqueeze(3).broadcast_to([128, 2, NC, D])
            a_t = fin_pool.tile([128, 2, NC, D], F32, tag="a")
            nc.vector.scalar_tensor_tensor(
                a_t[:], p_o_pair[:, :, :, 0:D], 0.0, r4b,
                mybir.AluOpType.min, mybir.AluOpType.mult,
            )
            v_t = fin_pool.tile([128, 2, NC, D], F32, tag="v")
            nc.scalar.activation(
                v_t[:], a_t[:], mybir.ActivationFunctionType.Exp
            )
            b_t = fin_pool.tile([128, 2, NC, D], F32, tag="b")
            nc.vector.scalar_tensor_tensor(
                b_t[:], p_o_pair[:, :, :, 0:D], 0.0, r4b,
                mybir.AluOpType.max, mybir.AluOpType.mult,
            )
            o_t = out_pool.tile([128, 2, NC, D], F32, tag="o")
            nc.vector.scalar_tensor_tensor(
                o_t[:], b_t[:], -1.0, v_t[:],
                mybir.AluOpType.add, mybir.AluOpType.add,
            )
            dst = out_ap[ds(g - 1, 2)].transpose([2, 0, 1, 3])
            nc.sync.dma_start(dst, o_t[:])

        def emit_final_one(hh, gg):
            # final for one graph of the pair (tail-drain trim)
            r4 = fin_pool.tile([128, 2, NC], F32, tag="r4")
            nc.vector.reciprocal(r4[:, 0, :], p_o_pair[:, hh, :, D + 1])
            r4b = r4[:, 0, :].unsqueeze(2).broadcast_to([128, NC, D])
            a_t = fin_pool.tile([128, 2, NC, D], F32, tag="a")
            nc.vector.scalar_tensor_tensor(
                a_t[:, 0], p_o_pair[:, hh, :, 0:D], 0.0, r4b,
                mybir.AluOpType.min, mybir.AluOpType.mult,
            )
            v_t = fin_pool.tile([128, 2, NC, D], F32, tag="v")
            nc.scalar.activation(
                v_t[:, 0], a_t[:, 0], mybir.ActivationFunctionType.Exp
            )
            b_t = fin_pool.tile([128, 2, NC, D], F32, tag="b")
            nc.vector.scalar_tensor_tensor(
                b_t[:, 0], p_o_pair[:, hh, :, 0:D], 0.0, r4b,
                mybir.AluOpType.max, mybir.AluOpType.mult,
            )
            o_t = out_pool.tile([128, 2, NC, D], F32, tag="o")
            nc.vector.scalar_tensor_tensor(
                o_t[:, 0], b_t[:, 0], -1.0, v_t[:, 0],
                mybir.AluOpType.add, mybir.AluOpType.add,
            )
            dst = out_ap[gg].transpose([1, 0, 2])
            nc.sync.dma_start(dst, o_t[:, 0])

        for mc in range(NC):
            for c in range(NC):
                nc.tensor.matmul(
                    p_o_pair[:, half, mc, 0 : D + 2],
                    p_t[:, c, ds(mc * 128, 128)],
                    wha[:, c, :],
                    start=(c == 0),
                    stop=(c == NC - 1),
                )
        # Last pair: per-graph finals so graph G-2's ELU/DMA overlaps
        # graph G-1's attention matmuls (tail-drain trim).
        if g >= G - 2:
            emit_final_one(half, g)
        elif half == 1:
            emit_final_pair()


_CACHE = {}


def _build():
    if "nc" in _CACHE:
        return _CACHE["nc"]
    nc = bacc.Bacc(
        "TRN2", target_bir_lowering=False, debug=False, num_devices=N_CORES
    )
    ht_d = nc.dram_tensor("ht", [G, IN_DIM, M], BF16, kind="ExternalInput")
    wq_d = nc.dram_tensor("wq", [IN_DIM, D + 1], BF16, kind="ExternalInput")
    wsb_d = nc.dram_tensor("wsb", [IN_DIM, 128], BF16, kind="ExternalInput")
    # [G, NC, 128, D] has the same memory layout as [G, M, D] (M = NC*128)
    out_d = nc.dram_tensor("out", [G, NC, 128, D], F32, kind="ExternalOutput")
    with tile.TileContext(nc) as tc:
        _gat_body(tc, out_d.ap(), ht_d.ap(), wq_d.ap(), wsb_d.ap())
    nc.compile()
    _CACHE["nc"] = nc
    return nc


def kernel(h, W, a):
    global LAST_RESULTS
    h = np.asarray(h, dtype=np.float32)
    W = np.asarray(W, dtype=np.float32)
    a = np.asarray(a, dtype=np.float32)

    wt = W.T.astype(np.float32)  # [128, 64]
    wq = np.concatenate(
        [wt, (wt @ a[D:])[:, None]], axis=1
    ).astype(ml_dtypes.bfloat16)  # [128, 65] = [W.T | w_dst]
    w_src = (wt @ a[:D]).astype(ml_dtypes.bfloat16)  # [128]
    wsb = np.ascontiguousarray(np.repeat(w_src[:, None], 128, axis=1))

    nc = _build()
    in_maps = []
    for c in range(N_CORES):
        h_c = h[c * G : (c + 1) * G]  # [G, 512, 128]
        ht_c = np.ascontiguousarray(h_c.transpose(0, 2, 1)).astype(
            ml_dtypes.bfloat16
        )  # [G, 128, 512]
        in_maps.append({"ht": ht_c, "wq": wq, "wsb": wsb})

    res = run_bass_kernel_spmd(nc, in_maps, list(range(N_CORES)))
    LAST_RESULTS = res
    out = np.concatenate(
        [r["out"].reshape(G, M, D) for r in res.results], axis=0
    )
    return out.astype(np.float32)
